# revision 16
# baseline (speedup 1.0000x reference)
"""NeighborAttention (B=4, N=4096, K=32, C=128, H=4) on 8 Trainium2 cores.

Data-parallel over the flattened (B*N) node axis; weights replicated.
Channel-major layout [row (4d+h), node-major free].  All heavy tensors are
bf16; matmuls run at 1 cycle/row.

Mask-aware bucketing: attention is permutation-invariant over the K
neighbors, and masked neighbors are zeroed.  The host packs each node's
unmasked neighbors first, rounds the count up to a bucket width
Kb in {8,12,16,20,24,28,32}, sorts nodes by bucket, and deals them
round-robin to the 8 cores so every core sees identical bucket counts
(padded by at most 7 dummy nodes).  Since E[cnt]=16, this drops ~45% of
all columns from every engine.  Padded slots have et=0, so they score 0
and contribute exp(0)=1 to the softmax denominator; the host sends the
per-node count correction (Kb - cnt) to subtract.

Per piece (<=1024 cols):
  KT   = WK' @ ET            (PE, 512-col matmuls -> 2-bank PSUM)
  prod = KT * bcast_j(QT)    (DVE 1x: fp32 PSUM operand)
  srep = Hrep @ prod         (PE)   head-summed scores, replicated over d
  e    = exp(srep)           (ACT -> bf16 SBUF; shift-invariance makes
                              max-subtraction unnecessary at these scales)
  VT   = WV' @ ET            (PE)
  v    = copy(VT)            (ACT -> bf16 SBUF; enables 2x DVE below)
  uv   = e * v               (DVE 2x)
Per chunk (<=8192 cols): pairwise bf16 trees on DVE
  usum = sum_j uv, umax = max_j uv, z = sum_j e
Epilogue: z -= (Kb - cnt), rz = exp(-ln(z)) on ACT,
  out = (WO_mean+WO_sum)' @ (usum*rz) + WO_max' @ (umax*rz).
attn sums to exactly 1, so aggr_mean == aggr_sum and the W_O blocks fold.
"""
import numpy as np
import ml_dtypes
import concourse.bass as bass
import concourse.bacc as bacc
import concourse.mybir as mybir
from concourse import tile
from concourse.bass_utils import run_bass_kernel_spmd

F32 = mybir.dt.float32
BF16 = mybir.dt.bfloat16
ALU = mybir.AluOpType
AF = mybir.ActivationFunctionType

K = 32
C = 128
H = 4
D = 32
NCORES = 8

BUCKETS = (8, 12, 16, 20, 24, 28, 32)
CHUNK_COLS = 8192
PIECE_COLS = 1024
MM = 512

_NC_CACHE = {}


def _tree_seg(nc, tmps, src, nn, w, out_f32, op):
    """Pairwise-reduce src [C, nn*w] windows of w -> out_f32 [C, nn]."""
    cur = src[:, :nn * w].rearrange("p (n j) -> p n j", j=w)
    li = 0
    while w > 2:
        h, odd = w // 2, w % 2
        wout = h + odd
        tt = tmps[li % len(tmps)]
        assert tt.shape[1] >= nn * wout, (nn, wout)
        t = tt[:, :nn * wout].rearrange("p (n j) -> p n j", j=wout)
        nc.vector.tensor_tensor(t[:, :, 0:h], cur[:, :, 0:h],
                                cur[:, :, h:2 * h], op=op)
        if odd:
            nc.vector.tensor_copy(t[:, :, h:h + 1], cur[:, :, 2 * h:2 * h + 1])
        cur = t
        w = wout
        li += 1
    nc.vector.tensor_tensor(out_f32.unsqueeze(2), cur[:, :, 0:1],
                            cur[:, :, 1:2], op=op)


def build_nc(nloc_pad, segments):
    """segments: tuple of (Kb, n_nodes) with sum(n_nodes) == nloc_pad."""
    key = (nloc_pad, segments)
    if key in _NC_CACHE:
        return _NC_CACHE[key]
    total_cols = sum(kb * nn for kb, nn in segments)

    nc = bacc.Bacc()
    et = nc.dram_tensor("et", [C, total_cols], BF16, kind="ExternalInput")
    xt = nc.dram_tensor("xt", [C, nloc_pad], BF16, kind="ExternalInput")
    wqt = nc.dram_tensor("wqt", [C, C], BF16, kind="ExternalInput")
    wkt = nc.dram_tensor("wkt", [C, C], BF16, kind="ExternalInput")
    wvt = nc.dram_tensor("wvt", [C, C], BF16, kind="ExternalInput")
    hrep = nc.dram_tensor("hrep", [C, C], BF16, kind="ExternalInput")
    wost = nc.dram_tensor("wost", [C, C], BF16, kind="ExternalInput")
    wo3t = nc.dram_tensor("wo3t", [C, C], BF16, kind="ExternalInput")
    identd = nc.dram_tensor("ident", [C, C], BF16, kind="ExternalInput")
    mcorr = nc.dram_tensor("mcorr", [C, nloc_pad], BF16, kind="ExternalInput")
    out = nc.dram_tensor("out", [C, nloc_pad], F32, kind="ExternalOutput")

    with tile.TileContext(nc) as tc:
        with tc.tile_pool(name="wts", bufs=1) as wpool, \
             tc.tile_pool(name="xin", bufs=1) as xpool, \
             tc.tile_pool(name="etp", bufs=2) as etpool, \
             tc.tile_pool(name="qp", bufs=2) as qpool, \
             tc.tile_pool(name="pp", bufs=2) as ppool, \
             tc.tile_pool(name="vp", bufs=2) as vpool, \
             tc.tile_pool(name="ep", bufs=2) as epool, \
             tc.tile_pool(name="uvp", bufs=2) as uvpool, \
             tc.tile_pool(name="tp", bufs=1) as tpool, \
             tc.tile_pool(name="acc", bufs=1) as accp, \
             tc.tile_pool(name="epi", bufs=1) as epip, \
             tc.tile_pool(name="outp", bufs=1) as outp, \
             tc.tile_pool(name="pkt", bufs=2, space="PSUM") as pkt, \
             tc.tile_pool(name="pvt", bufs=1, space="PSUM") as pvt, \
             tc.tile_pool(name="psr", bufs=1, space="PSUM") as psr:

            w_q = wpool.tile([C, C], BF16, tag="wq")
            w_k = wpool.tile([C, C], BF16, tag="wk")
            w_v = wpool.tile([C, C], BF16, tag="wv")
            w_h = wpool.tile([C, C], BF16, tag="wh")
            w_os = wpool.tile([C, C], BF16, tag="wos")
            w_o3 = wpool.tile([C, C], BF16, tag="wo3")
            w_id = wpool.tile([C, C], BF16, tag="wid")
            nc.sync.dma_start(w_id[:], identd[:])
            nc.sync.dma_start(w_q[:], wqt[:])
            nc.sync.dma_start(w_k[:], wkt[:])
            nc.sync.dma_start(w_v[:], wvt[:])
            nc.sync.dma_start(w_h[:], hrep[:])
            nc.sync.dma_start(w_os[:], wost[:])
            nc.sync.dma_start(w_o3[:], wo3t[:])

            xt_sb = xpool.tile([C, nloc_pad], BF16, tag="xt")
            nc.sync.dma_start(xt_sb[:], xt[:])
            mc_sb = xpool.tile([C, nloc_pad], BF16, tag="mc")
            nc.sync.dma_start(mc_sb[:], mcorr[:])

            usum_c = accp.tile([C, nloc_pad], F32, tag="usum")
            umax_c = accp.tile([C, nloc_pad], F32, tag="umax")
            z_c = accp.tile([C, nloc_pad], F32, tag="zc")

            tr0 = tpool.tile([C, 4096], BF16, tag="t0")
            tr1 = tpool.tile([C, 2048], BF16, tag="t1")
            tr2 = tpool.tile([C, 2048], BF16, tag="t2")
            tmps = [tr0, tr1, tr2]

            node_off = 0
            col_off = 0
            for kb, seg_nodes in segments:
                chunk_n = CHUNK_COLS // kb
                piece_n = PIECE_COLS // kb
                for ch0 in range(0, seg_nodes, chunk_n):
                    nn = min(chunk_n, seg_nodes - ch0)
                    ccols = nn * kb
                    n0 = node_off + ch0
                    c0 = col_off + ch0 * kb

                    et_sb = etpool.tile([C, CHUNK_COLS], BF16, tag="et")
                    nc.sync.dma_start(et_sb[:, :ccols], et[:, c0:c0 + ccols])

                    q_ps = pkt.tile([C, PIECE_COLS], F32, tag="kt")
                    nc.tensor.matmul(q_ps[:, :nn], w_q[:],
                                     xt_sb[:, n0:n0 + nn],
                                     start=True, stop=True)
                    q_sb = qpool.tile([C, 1024], BF16, tag="q")
                    nc.scalar.activation(q_sb[:, :nn], q_ps[:, :nn], AF.Copy)

                    e_ch = epool.tile([C, CHUNK_COLS], BF16, tag="e")
                    uv_ch = uvpool.tile([C, CHUNK_COLS], BF16, tag="uv")

                    for p0 in range(0, nn, piece_n):
                        pnn = min(piece_n, nn - p0)
                        pc = pnn * kb          # cols in piece
                        pc0 = p0 * kb          # col offset in chunk

                        kt_ps = pkt.tile([C, PIECE_COLS], F32, tag="kt")
                        s = min(MM, pc)
                        nc.tensor.matmul(kt_ps[:, :s], w_k[:],
                                         et_sb[:, pc0:pc0 + s],
                                         start=True, stop=True)
                        if pc > MM:
                            nc.tensor.matmul(kt_ps[:, MM:pc], w_k[:],
                                             et_sb[:, pc0 + MM:pc0 + pc],
                                             start=True, stop=True)

                        prod = ppool.tile([C, PIECE_COLS], BF16, tag="prod")
                        qb = q_sb[:, p0:p0 + pnn].unsqueeze(2).broadcast_to(
                            (C, pnn, kb))
                        nc.vector.tensor_mul(
                            prod[:, :pc].rearrange("p (n j) -> p n j", j=kb),
                            kt_ps[:, :pc].rearrange("p (n j) -> p n j", j=kb),
                            qb)

                        sr_ps = psr.tile([C, PIECE_COLS], F32, tag="sr")
                        nc.tensor.matmul(sr_ps[:, :s], w_h[:], prod[:, :s],
                                         start=True, stop=True)
                        if pc > MM:
                            nc.tensor.matmul(sr_ps[:, MM:pc], w_h[:],
                                             prod[:, MM:pc],
                                             start=True, stop=True)
                        nc.scalar.activation(e_ch[:, pc0:pc0 + pc],
                                             sr_ps[:, :pc], AF.Exp)

                        vt_ps = pvt.tile([C, PIECE_COLS], F32, tag="vt")
                        nc.tensor.matmul(vt_ps[:, :s], w_v[:],
                                         et_sb[:, pc0:pc0 + s],
                                         start=True, stop=True)
                        if pc > MM:
                            nc.tensor.matmul(vt_ps[:, MM:pc], w_v[:],
                                             et_sb[:, pc0 + MM:pc0 + pc],
                                             start=True, stop=True)
                        v_sb = vpool.tile([C, PIECE_COLS], BF16, tag="v")
                        nc.scalar.activation(v_sb[:, :pc], vt_ps[:, :pc],
                                             AF.Copy)

                        nc.vector.tensor_mul(uv_ch[:, pc0:pc0 + pc],
                                             e_ch[:, pc0:pc0 + pc],
                                             v_sb[:, :pc])

                    _tree_seg(nc, tmps, uv_ch, nn, kb,
                              usum_c[:, n0:n0 + nn], ALU.add)
                    _tree_seg(nc, tmps, uv_ch, nn, kb,
                              umax_c[:, n0:n0 + nn], ALU.max)
                    # z = sum_j e via PE identity-accumulation (frees DVE)
                    z_ps = pkt.tile([C, PIECE_COLS], F32, tag="kt")
                    e3 = e_ch[:, :nn * kb].rearrange("p (n j) -> p n j", j=kb)
                    for j in range(kb):
                        nc.tensor.matmul(z_ps[:, :nn], w_id[:], e3[:, :, j],
                                         start=(j == 0), stop=(j == kb - 1))
                    nc.scalar.activation(z_c[:, n0:n0 + nn], z_ps[:, :nn],
                                         AF.Copy)

                node_off += seg_nodes
                col_off += seg_nodes * kb

            # epilogue, blocked so DVE/ACT/PE/DMA pipeline across blocks
            ztmp = epip.tile([C, nloc_pad], F32, tag="ztmp")
            rz = epip.tile([C, nloc_pad], F32, tag="rz")
            wsn = epip.tile([C, nloc_pad], BF16, tag="wsn")
            mxn = epip.tile([C, nloc_pad], BF16, tag="mxn")
            out_sb = outp.tile([C, nloc_pad], F32, tag="osb")
            for b0 in range(0, nloc_pad, MM):
                ob = min(MM, nloc_pad - b0)
                sl = slice(b0, b0 + ob)
                nc.vector.tensor_sub(ztmp[:, sl], z_c[:, sl], mc_sb[:, sl])
                # fully-masked nodes: usum/umax rows are exactly 0; any
                # finite 1/z gives the correct 0 output — just avoid inf*0.
                nc.vector.tensor_scalar_max(ztmp[:, sl], ztmp[:, sl], 1e-20)
                # 1/z = exp(-ln(z)): Ln and Exp share one ACT table set;
                # bass blocks the Reciprocal ACT function.
                nc.scalar.activation(rz[:, sl], ztmp[:, sl], AF.Ln)
                nc.scalar.activation(ztmp[:, sl], rz[:, sl], AF.Exp,
                                     scale=-1.0)
                nc.vector.tensor_mul(wsn[:, sl], usum_c[:, sl], ztmp[:, sl])
                nc.vector.tensor_mul(mxn[:, sl], umax_c[:, sl], ztmp[:, sl])
                o_ps = psr.tile([C, PIECE_COLS], F32, tag="sr")
                nc.tensor.matmul(o_ps[:, :ob], w_os[:], wsn[:, sl],
                                 start=True, stop=False)
                nc.tensor.matmul(o_ps[:, :ob], w_o3[:], mxn[:, sl],
                                 start=False, stop=True)
                nc.scalar.activation(out_sb[:, sl], o_ps[:, :ob], AF.Copy)
                nc.sync.dma_start(out[:, sl], out_sb[:, sl])

    nc.compile()
    _NC_CACHE[key] = nc
    return nc


def _perm_dh(w):
    """[(h*32+d), cin] -> [cin, (4d+h)] in bf16"""
    wt = np.asarray(w, dtype=np.float32).reshape(H, D, -1)
    return np.ascontiguousarray(
        np.transpose(wt, (2, 1, 0)).reshape(-1, H * D)).astype(
            ml_dtypes.bfloat16)


def prep_inputs(h_X, h_E, mask_attn, W_Q, W_K, W_V, W_O):
    h_X = np.asarray(h_X, dtype=np.float32)
    h_E = np.asarray(h_E, dtype=np.float32)
    mask_attn = np.asarray(mask_attn)
    W_Q = np.asarray(W_Q, dtype=np.float32)
    W_K = np.asarray(W_K, dtype=np.float32)
    W_V = np.asarray(W_V, dtype=np.float32)
    W_O = np.asarray(W_O, dtype=np.float32)

    B, N, Kn, Cin = h_E.shape
    BN = B * N

    maskf = mask_attn.astype(np.float32).reshape(BN, Kn)
    ef = h_E.reshape(BN, Kn, Cin)
    xf = h_X.reshape(BN, -1)
    cnt = maskf.sum(axis=1).astype(np.int64)

    # bucket per node, neighbor packing order (unmasked first, stable)
    barr = np.asarray(BUCKETS)
    bidx = np.searchsorted(barr, cnt)          # index of smallest Kb >= cnt
    perm_j = np.argsort(-maskf, axis=1, kind="stable")

    # sort nodes by bucket (stable), deal round-robin to cores
    order = np.argsort(bidx, kind="stable")
    core_ids = [order[i::NCORES] for i in range(NCORES)]
    # per-core per-bucket counts; pad to max over cores
    nb = np.zeros((NCORES, len(BUCKETS)), np.int64)
    for i in range(NCORES):
        nb[i] = np.bincount(bidx[core_ids[i]], minlength=len(BUCKETS))
    nb_max = nb.max(axis=0)
    segments = tuple((int(barr[b]), int(nb_max[b]))
                     for b in range(len(BUCKETS)) if nb_max[b] > 0)
    nloc_pad = int(nb_max.sum())
    total_cols = sum(kb * nn for kb, nn in segments)

    wqt = _perm_dh(W_Q / np.sqrt(D))
    wkt = _perm_dh(W_K)
    wvt = _perm_dh(W_V)

    idx = np.arange(C)
    hh = idx % H
    hrep = (hh[:, None] == hh[None, :]).astype(ml_dtypes.bfloat16)

    wos = W_O[:, :C] + W_O[:, C:2 * C]
    wo3 = W_O[:, 2 * C:]
    wost = np.ascontiguousarray(
        wos.T.reshape(H, D, C).transpose(1, 0, 2).reshape(C, C)).astype(
            ml_dtypes.bfloat16)
    wo3t = np.ascontiguousarray(
        wo3.T.reshape(H, D, C).transpose(1, 0, 2).reshape(C, C)).astype(
            ml_dtypes.bfloat16)

    in_maps = []
    ids_padded_all = []
    for i in range(NCORES):
        ids = core_ids[i]
        etc = np.zeros((C, total_cols), ml_dtypes.bfloat16)
        xtc = np.zeros((C, nloc_pad), ml_dtypes.bfloat16)
        mcc = np.zeros((C, nloc_pad), ml_dtypes.bfloat16)
        ids_padded = np.full(nloc_pad, -1, np.int64)
        no = 0
        co = 0
        for b, (kb, nn_seg) in zip(
                [b for b in range(len(BUCKETS)) if nb_max[b] > 0], segments):
            sel = ids[bidx[ids] == b]
            nsel = len(sel)
            if nsel:
                pj = perm_j[sel][:, :kb]                      # [nsel, kb]
                g = np.take_along_axis(ef[sel], pj[:, :, None], axis=1)
                gm = np.take_along_axis(maskf[sel], pj, axis=1)
                g = g * gm[:, :, None]                        # [nsel, kb, C]
                etc[:, co:co + nsel * kb] = g.reshape(
                    nsel * kb, Cin).T.astype(ml_dtypes.bfloat16)
                xtc[:, no:no + nsel] = xf[sel].T.astype(ml_dtypes.bfloat16)
                mcc[:, no:no + nsel] = np.broadcast_to(
                    kb - cnt[sel], (C, nsel)).astype(ml_dtypes.bfloat16)
                ids_padded[no:no + nsel] = sel
            # padded dummy nodes: et/x zero, correction = kb so z-mc = 0
            if nn_seg > nsel:
                mcc[:, no + nsel:no + nn_seg] = np.float32(kb)
            no += nn_seg
            co += nn_seg * kb
        ids_padded_all.append(ids_padded)
        in_maps.append({
            "et": etc, "xt": xtc,
            "wqt": wqt, "wkt": wkt, "wvt": wvt, "hrep": hrep,
            "wost": wost, "wo3t": wo3t, "mcorr": mcc,
            "ident": np.eye(C, dtype=ml_dtypes.bfloat16),
        })
    meta = {"segments": segments, "nloc_pad": nloc_pad,
            "ids_padded": ids_padded_all}
    return in_maps, meta


def assemble_output(results, B, N, meta):
    BN = B * N
    outf = np.empty((BN, C), np.float32)
    for i, r in enumerate(results):
        ids = meta["ids_padded"][i]
        valid = ids >= 0
        outf[ids[valid]] = r["out"].T[valid]
    return outf.reshape(B, N, C)


def kernel(h_X, h_E, mask_attn, W_Q, W_K, W_V, W_O):
    in_maps, meta = prep_inputs(h_X, h_E, mask_attn, W_Q, W_K, W_V, W_O)
    nc = build_nc(meta["nloc_pad"], meta["segments"])
    res = run_bass_kernel_spmd(nc, in_maps, core_ids=list(range(NCORES)))
    B, N = h_X.shape[0], h_X.shape[1]
    return assemble_output(res.results, B, N, meta)
